# revision 6
# baseline (speedup 1.0000x reference)
"""Trainium2 Bass kernel for the siamese-kNN classification head.

Reference computation (B=256, N=2000, D=512, C=100):
    scores[b,n] = sigmoid(sum_d w_d * |a[b,d] - S[n,d]| + kb)
    out[b,c]    = (scores @ L)[b,c] / count_c     (0 where count_c == 0)

Strategy
--------
Data-parallel over the batch: core i handles rows 32*i .. 32*i+32, no
collectives.  The pairwise |a-s| volume (32*2000*512 element-touches per
core) dominates; |x| = relu(2x) - x splits each score into a nonlinear
slab relu(A''-S'') (A''=2|w|a, S''=2|w|S) that one engine produces in a
single instruction per [128,2000] d-chunk, plus a separable linear part
folded into a rank-2 f32r correction matmul.  The PE reduces each slab
over d with a sliding-window +-sign stationary (col b of a [128,32]
window = sign, rest zero).

The baseline was PE-ingest-bound (1 bf16 column/cycle -> ~110us/core).
This version converts most slabs to fp8e4 and reduces PAIRS of them in
one DoubleRow matmul (stationary [128,2,32], moving [128,2,seg], 0.5
cycles/column -> 2x PE throughput).  Slab production is spread over
two engines, batch rows assigned per class:
  - DVB rows: DVE tensor_scalar bf16 slabs (2x DVE mode), bf16 matmul.
  - DV8 rows: DVE fp8 slabs (1x mode), DoubleRow pairs (chunks 01/23).
  - AC8 rows: ACT activation(Relu) fp8 slabs, DoubleRow pairs.
The PE instruction stream is ordered by estimated slab-ready times so
no engine stalls the pipe.  fp8e4 quantization of the slab keeps max
rel err ~1.5e-2 (< 2e-2 gate); DVB rows stay bf16-accurate.

DoubleRow ISA restrictions honored: sign-window planes have stride 64
(even, 16B-aligned), moving-plane stride N=2000, all fp8 offsets even,
PSUM dst starts at partition 0.
"""

import sys

for _p in ("/opt/trn_rl_repo", "/root/.axon_site/_ro/trn_rl_repo"):
    if _p not in sys.path:
        sys.path.append(_p)

import numpy as np

B, N, D, C = 256, 2000, 512, 100
NP = 2048                  # label rows padded to 16 full chunks
NCORES = 8
BSH = B // NCORES          # 32 batch rows per core
DCH = D // 128             # 4 d-chunks
NSEG = 4                   # PSUM free-dim segments
SEG = N // NSEG            # 500
NLAB = NP // 128           # 16 label chunks

# batch-row classes (per core): counts must sum to BSH
# (GPSIMD is NOT used: its tensor_scalar runs ~15ns/elem on HW and poisons
# concurrent DVE/ACT throughput via SBUF contention)
N_DVB = 11                 # DVE bf16 rows
N_DV8 = 10                 # DVE fp8 rows
N_AC8 = 11                 # ACT fp8 rows
assert N_DVB + N_DV8 + N_AC8 == BSH

DVB_ROWS = list(range(0, N_DVB))
DV8_ROWS = list(range(N_DVB, N_DVB + N_DV8))
AC8_ROWS = list(range(N_DVB + N_DV8, BSH))

_CACHE = {}


def _split_multi_waits(nc):
    """TRN2 TPB instructions encode at most ONE semaphore wait, but Tile can
    attach several (e.g. the tail drain, or an op whose inputs arrived on two
    DMA queues); this walrus build refuses those.  Splitting the extras into
    single-wait NOPs directly before the instruction on the same engine is
    semantically identical (engines execute their block instructions in
    order)."""
    from concourse import mybir

    for fn in nc.m.functions:
        for bb in fn.blocks:
            out = []
            for inst in bb.instructions:
                si = inst.sync_info
                if si is not None and si.on_wait and len(si.on_wait) > 1:
                    waits = list(si.on_wait)
                    for j, w in enumerate(waits[:-1]):
                        out.append(mybir.InstNoOp(
                            name=f"{inst.name}-sw{j}", engine=inst.engine,
                            sync_info=mybir.SyncInfo(on_wait=[w], on_update=[]),
                            ins=[], outs=[]))
                    inst.sync_info = mybir.SyncInfo(
                        on_wait=[waits[-1]], on_update=list(si.on_update))
                out.append(inst)
            bb.instructions = out


C_DVB, C_DV8, C_AC8 = 0.74, 1.26, 1.95
DVB_PREFIX = 8             # pure-bf16 DVE warmup before fp8 pairs interleave
CH_AVAIL = [2.0, 3.5, 6.5, 8.0]


def _dve_order():
    """DVE emission order: a pure-bf16 warmup burst (so the PE never starves
    while ACT/fp8 production ramps), then DVB slabs with DV8 pairs injected
    at an even rate."""
    dvb = [('bf16', ch, b) for ch in range(DCH) for b in DVB_ROWS]
    dv8 = [('pair', 'dv8', cp, b) for b in DV8_ROWS for cp in range(2)]
    out = dvb[:DVB_PREFIX]
    rest = dvb[DVB_PREFIX:]
    npair = len(dv8)
    di = 0
    for k in range(npair):
        take = (len(rest) * (k + 1)) // npair - (len(rest) * k) // npair
        out.extend(rest[di : di + take])
        di += take
        out.append(dv8[k])
    out.extend(rest[di:])
    return out


def _schedule():
    """Estimated-ready-time ordering of PE work items.

    Items: ('bf16', ch, b) one bf16 slab; ('pair', cls, chpair, b) one fp8
    DoubleRow pair; ('corr',) the rank-2 correction.  Producer timelines use
    measured per-slab costs (us) and DMA chunk-availability floors."""
    items = [(4.0, 0, ('corr',))]
    seq = 1

    t = 0.0
    for it in _dve_order():
        if it[0] == 'bf16':
            t = max(t, CH_AVAIL[it[1]]) + C_DVB
        else:
            t = max(t, CH_AVAIL[2 * it[2] + 1]) + 2 * C_DV8
        items.append((t, seq, it))
        seq += 1

    t = 0.0
    for b in AC8_ROWS:
        for cp in range(2):
            t = max(t, CH_AVAIL[2 * cp + 1]) + 2 * C_AC8
            items.append((t, seq, ('pair', 'ac8', cp, b)))
            seq += 1

    items.sort(key=lambda x: (x[0], x[1]))
    return [it for _, _, it in items]


def _build_nc():
    import concourse.bass as bass
    import concourse.tile as tile
    from concourse import mybir

    f32 = mybir.dt.float32
    f32r = mybir.dt.float32r
    bf16 = mybir.dt.bfloat16
    fp8 = mybir.dt.float8e4
    nc = bass.Bass()

    s2t_d = nc.declare_dram_parameter("s2t", [D, N], bf16, isOutput=False)
    a2t_d = nc.declare_dram_parameter("a2t", [DCH, 128, BSH], f32, isOutput=False)
    # bf16 sliding-window sign tile: zero except col 31 = -sign per chunk
    sgnn_d = nc.declare_dram_parameter("sgnn", [128, DCH, 63], bf16, isOutput=False)
    # fp8 sign tiles [128, 2, DCH, 64]: [:,0]= -sign (DVE/GPS), [:,1]= +sign (ACT)
    sgn8_d = nc.declare_dram_parameter("sgn8", [128, 2, DCH, 64], fp8, isOutput=False)
    # corr[0] = clhs [2, BSH], corr[1] = crhs [2, N] combined
    corr_d = nc.declare_dram_parameter("corr", [2, BSH + N], f32r, isOutput=False)
    # labels packed [128, chunk, C] (host-padded to 2048 rows), bf16 (0/1 exact)
    lab_d = nc.declare_dram_parameter("labels", [128, NLAB, C], bf16, isOutput=False)
    ident_d = nc.declare_dram_parameter("ident", [32, 32], bf16, isOutput=False)
    recb_d = nc.declare_dram_parameter("recb", [BSH, C], f32, isOutput=False)
    out_d = nc.declare_dram_parameter("out", [BSH, C], f32, isOutput=True)

    with tile.TileContext(nc) as tc:
        with (
            tc.tile_pool(name="const", bufs=1) as const,
            tc.tile_pool(name="dslab", bufs=8) as dpool,
            tc.tile_pool(name="v8", bufs=4) as v8pool,
            tc.tile_pool(name="a8", bufs=4) as a8pool,
            tc.tile_pool(name="bank", bufs=8, space="PSUM") as bankp,
        ):
            # ---- constant loads, issued in PARALLEL across engine
            # sequencers (a serial stream of dma_starts on Sync costs
            # ~600-700ns EACH and delayed first compute to ~15us).  Sync
            # carries the critical s2t chunks; the idle Pool sequencer
            # issues every small constant (~25ns each); ACT issues the
            # late-needed s2t chunk pair before its first slab.
            s2t0 = const.tile([128, N], bf16, name="s2t0", tag="s2t0")
            nc.sync.dma_start(s2t0[:], s2t_d[0:128, :])
            s2t1 = const.tile([128, N], bf16, name="s2t1", tag="s2t1")
            nc.sync.dma_start(s2t1[:], s2t_d[128:256, :])
            a2t = const.tile([128, DCH * BSH], f32, name="a2t", tag="a2t")
            nc.gpsimd.dma_start(
                a2t[:].rearrange("p (c b) -> p c b", c=DCH),
                a2t_d[:].rearrange("c p b -> p c b"),
            )
            sgnn = const.tile([128, DCH, 63], bf16, name="sgnn", tag="sgnn")
            nc.gpsimd.dma_start(sgnn[:], sgnn_d[:])
            sgn8 = const.tile([128, 2, DCH, 64], fp8, name="sgn8", tag="sgn8")
            nc.gpsimd.dma_start(sgn8[:], sgn8_d[:])
            corr = const.tile([2, BSH + N], f32r, name="corr", tag="corr")
            nc.gpsimd.dma_start(corr[:], corr_d[:])
            s2t23 = const.tile([128, 2 * N], bf16, name="s2t23", tag="s2t23")
            nc.scalar.dma_start(
                s2t23[:].rearrange("p (c n) -> p c n", c=2),
                s2t_d[:].rearrange("(c p) n -> p c n", p=128)[:, 2:4, :],
            )
            labs = const.tile([128, NLAB, C], bf16, name="labs", tag="labs")
            nc.gpsimd.dma_start(labs[:], lab_d[:])
            ident = const.tile([32, 32], bf16, name="ident", tag="ident")
            nc.gpsimd.dma_start(ident[:], ident_d[:])
            recb = const.tile([BSH, C], f32, name="recb", tag="recb")
            nc.gpsimd.dma_start(recb[:], recb_d[:])
            s2t = [s2t0, s2t1] + [
                s2t23[:, ch * N : (ch + 1) * N] for ch in range(2)
            ]
            clhs = corr[:, :BSH]
            crhs = corr[:, BSH:]

            psc = [
                bankp.tile([BSH, SEG], f32, name=f"psc{s}", tag="bank")
                for s in range(NSEG)
            ]

            # ---- producer emission (per-engine program order must match the
            # schedule's timeline assumptions)
            dvb_slab = {}
            pair_tiles = {}
            for it in _dve_order():
                if it[0] == 'bf16':
                    _, ch, b = it
                    slab = dpool.tile([128, N], bf16, name="dslab", tag="dslab")
                    nc.vector.tensor_scalar(
                        slab[:], s2t[ch][:],
                        a2t[:, ch * BSH + b : ch * BSH + b + 1], 0.0,
                        mybir.AluOpType.subtract, mybir.AluOpType.min,
                    )
                    dvb_slab[(ch, b)] = slab
                else:
                    _, _, cp, b = it
                    pt = v8pool.tile([128, 2, N], fp8, name="v8slab", tag="v8slab")
                    for i in range(2):
                        ch = 2 * cp + i
                        nc.vector.tensor_scalar(
                            pt[:, i, :], s2t[ch][:],
                            a2t[:, ch * BSH + b : ch * BSH + b + 1], 0.0,
                            mybir.AluOpType.subtract, mybir.AluOpType.min,
                        )
                    pair_tiles[('dv8', cp, b)] = pt
            for b in AC8_ROWS:
                for cp in range(2):
                    pt = a8pool.tile([128, 2, N], fp8, name="a8slab", tag="a8slab")
                    for i in range(2):
                        ch = 2 * cp + i
                        nc.scalar.activation(
                            pt[:, i, :], s2t[ch][:],
                            mybir.ActivationFunctionType.Relu,
                            bias=a2t[:, ch * BSH + b : ch * BSH + b + 1],
                            scale=-1.0,
                        )
                    pair_tiles[('ac8', cp, b)] = pt
            # preload the sigmoid ACT table while the PE drains (free insurance
            # against a ~1.3us table swap on the critical tail)
            sigwarm = const.tile([32, 1], bf16, name="sigwarm", tag="sigwarm")
            nc.scalar.activation(
                sigwarm[:], ident[:, 0:1],
                mybir.ActivationFunctionType.Sigmoid,
            )
            # ---- PE stream in estimated-ready order
            items = _schedule()
            started = False
            for idx, it in enumerate(items):
                last = idx == len(items) - 1
                start = not started
                started = True
                if it[0] == 'corr':
                    for s in range(NSEG):
                        nc.tensor.matmul(
                            psc[s][:], clhs,
                            crhs[:, SEG * s : SEG * (s + 1)],
                            start=start, stop=last, skip_group_check=True,
                        )
                elif it[0] == 'bf16':
                    _, ch, b = it
                    slab = dvb_slab[(ch, b)]
                    lhs = sgnn[:, ch, 31 - b : 63 - b]
                    for s in range(NSEG):
                        nc.tensor.matmul(
                            psc[s][:], lhs,
                            slab[:, SEG * s : SEG * (s + 1)],
                            start=start, stop=last, skip_group_check=True,
                        )
                else:
                    _, cls, cp, b = it
                    pt = pair_tiles[(cls, cp, b)]
                    k = 1 if cls == 'ac8' else 0
                    lhs = sgn8[:, k, 2 * cp : 2 * cp + 2, 31 - b : 63 - b]
                    for s in range(NSEG):
                        nc.tensor.matmul(
                            psc[s][:], lhs,
                            pt[:, :, SEG * s : SEG * (s + 1)],
                            start=start, stop=last, skip_group_check=True,
                            perf_mode=mybir.MatmulPerfMode.DoubleRow,
                        )

            # ---- sigmoid (PSUM -> SBUF, bf16) ----
            ssig = const.tile([BSH, N], bf16, name="ssig", tag="ssig")
            for s in range(NSEG):
                nc.scalar.activation(
                    ssig[:, SEG * s : SEG * (s + 1)], psc[s][:],
                    mybir.ActivationFunctionType.Sigmoid,
                )

            # ---- 16 transposes into ONE PSUM bank, one copy, final matmuls
            tpall = bankp.tile([128, NLAB * BSH], bf16, name="tpall", tag="bank")
            for k in range(NLAB):
                pk = min(128, N - 128 * k)
                nc.tensor.transpose(
                    tpall[:pk, BSH * k : BSH * k + BSH],
                    ssig[:, 128 * k : 128 * k + pk], ident[:],
                )
            sct = const.tile([128, NLAB * BSH], bf16, name="sct", tag="sct")
            nc.vector.tensor_copy(sct[:], tpall[:])
            out_ps = bankp.tile([BSH, C], f32, name="out_ps", tag="bank")
            for k in range(NLAB):
                pk = min(128, N - 128 * k)
                nc.tensor.matmul(
                    out_ps[:], sct[:pk, BSH * k : BSH * k + BSH],
                    labs[:pk, k, :],
                    start=(k == 0), stop=(k == NLAB - 1),
                )

            # ---- divide by counts, write out ----
            out_s = const.tile([BSH, C], f32, name="out_s", tag="out_s")
            nc.vector.tensor_mul(out_s[:], out_ps[:], recb[:])
            nc.sync.dma_start(out_d[:], out_s[:])

    _split_multi_waits(nc)
    return nc


def _prep_host(inputs, support_tensors, support_labels, kernel_w, kernel_b):
    import ml_dtypes

    bf16 = ml_dtypes.bfloat16
    e4m3 = ml_dtypes.float8_e4m3
    a = np.asarray(inputs, dtype=np.float32)
    S = np.asarray(support_tensors, dtype=np.float32)
    L = np.asarray(support_labels, dtype=np.float32)
    w = np.asarray(kernel_w, dtype=np.float32)
    kb = np.float32(np.asarray(kernel_b, dtype=np.float32))

    aw = 2.0 * np.abs(w)
    sgn = np.sign(w).astype(np.float32)
    s2t = np.ascontiguousarray((S * aw[None, :]).T).astype(bf16)   # [D, N]
    wS = (S @ w).astype(np.float32)                                # [N]
    wa = (a @ w).astype(np.float32)                                # [B]
    a2 = a * aw[None, :]                                           # [B, D]

    # sliding-window sign tiles: col 31 = -/+ sign chunk
    sgn_chunks = sgn.reshape(DCH, 128).T                           # [128, DCH]
    sgnn = np.zeros((128, DCH, 63), dtype=np.float32)
    sgnn[:, :, 31] = -sgn_chunks
    sgn8 = np.zeros((128, 2, DCH, 64), dtype=np.float32)
    sgn8[:, 0, :, 31] = -sgn_chunks
    sgn8[:, 1, :, 31] = sgn_chunks
    labp = np.zeros((NP, C), dtype=np.float32)
    labp[:N] = L
    labp = np.ascontiguousarray(
        labp.reshape(NLAB, 128, C).transpose(1, 0, 2)).astype(bf16)
    ident = np.eye(32, dtype=bf16)
    counts = L.sum(axis=0)
    recip = np.where(counts != 0, 1.0 / np.maximum(counts, 1e-30), 0.0)
    recb = np.broadcast_to(recip.astype(np.float32), (BSH, C)).copy()

    shared = {
        "s2t": s2t, "sgnn": sgnn.astype(bf16), "sgn8": sgn8.astype(e4m3),
        "labels": labp, "ident": ident, "recb": recb,
    }
    in_maps = []
    for c in range(NCORES):
        rows = slice(BSH * c, BSH * (c + 1))
        a2t_c = np.ascontiguousarray(
            a2[rows].T.reshape(DCH, 128, BSH))                     # [DCH,128,BSH]
        corr_c = np.empty((2, BSH + N), dtype=np.float32)
        corr_c[0, :BSH] = kb - wa[rows]
        corr_c[1, :BSH] = 1.0
        corr_c[0, BSH:] = 1.0
        corr_c[1, BSH:] = wS
        in_maps.append(dict(shared, a2t=a2t_c, corr=corr_c))
    return in_maps


def kernel(**inputs) -> np.ndarray:
    from concourse.bass_utils import run_bass_kernel_spmd

    if "nc" not in _CACHE:
        _CACHE["nc"] = _build_nc()
    nc = _CACHE["nc"]

    in_maps = _prep_host(
        inputs["inputs"], inputs["support_tensors"], inputs["support_labels"],
        inputs["kernel_w"], inputs["kernel_b"],
    )
    res = run_bass_kernel_spmd(nc, in_maps, list(range(NCORES)))
    return np.concatenate([res.results[i]["out"] for i in range(NCORES)], axis=0)


# revision 8
# speedup vs baseline: 1.1805x; 1.1805x over previous
"""Trainium2 Bass kernel for the siamese-kNN classification head.

Reference computation (B=256, N=2000, D=512, C=100):
    scores[b,n] = sigmoid(sum_d w_d * |a[b,d] - S[n,d]| + kb)
    out[b,c]    = (scores @ L)[b,c] / count_c     (0 where count_c == 0)

Strategy
--------
Data-parallel over the batch: core i handles rows 32*i .. 32*i+32, no
collectives.  The pairwise |a-s| volume (32*2000*512 element-touches per
core) dominates; |x| = relu(2x) - x splits each score into a nonlinear
slab relu(A''-S'') (A''=2|w|a, S''=2|w|S) that one engine produces in a
single instruction per [128,2000] d-chunk, plus a separable linear part
folded into a rank-2 f32r correction matmul.  The PE reduces each slab
over d with a sliding-window +-sign stationary (col b of a [128,32]
window = sign, rest zero).

The baseline was PE-ingest-bound (1 bf16 column/cycle -> ~110us/core).
This version converts most slabs to fp8e4 and reduces PAIRS of them in
one DoubleRow matmul (stationary [128,2,32], moving [128,2,seg], 0.5
cycles/column -> 2x PE throughput).  Slab production is spread over
two engines, batch rows assigned per class:
  - DVB rows: DVE tensor_scalar bf16 slabs (2x DVE mode), bf16 matmul.
  - DV8 rows: DVE fp8 slabs (1x mode), DoubleRow pairs (chunks 01/23).
  - AC8 rows: ACT activation(Relu) fp8 slabs, DoubleRow pairs.
The PE instruction stream is ordered by estimated slab-ready times so
no engine stalls the pipe.  fp8e4 quantization of the slab keeps max
rel err ~1.5e-2 (< 2e-2 gate); DVB rows stay bf16-accurate.

DoubleRow ISA restrictions honored: sign-window planes have stride 64
(even, 16B-aligned), moving-plane stride N=2000, all fp8 offsets even,
PSUM dst starts at partition 0.
"""

import sys

for _p in ("/opt/trn_rl_repo", "/root/.axon_site/_ro/trn_rl_repo"):
    if _p not in sys.path:
        sys.path.append(_p)

import numpy as np

B, N, D, C = 256, 2000, 512, 100
NP = 2048                  # label rows padded to 16 full chunks
NCORES = 8
BSH = B // NCORES          # 32 batch rows per core
DCH = D // 128             # 4 d-chunks
NSEG = 4                   # PSUM free-dim segments
SEG = N // NSEG            # 500
NLAB = NP // 128           # 16 label chunks

# batch-row classes (per core): counts must sum to BSH
# (GPSIMD is NOT used: its tensor_scalar runs ~15ns/elem on HW and poisons
# concurrent DVE/ACT throughput via SBUF contention)
N_DVB = 11                 # DVE bf16 rows
N_DV8 = 10                 # DVE fp8 rows
N_AC8 = 11                 # ACT fp8 rows
assert N_DVB + N_DV8 + N_AC8 == BSH

DVB_ROWS = list(range(0, N_DVB))
DV8_ROWS = list(range(N_DVB, N_DVB + N_DV8))
AC8_ROWS = list(range(N_DVB + N_DV8, BSH))

_CACHE = {}


def _split_multi_waits(nc):
    """TRN2 TPB instructions encode at most ONE semaphore wait, but Tile can
    attach several (e.g. the tail drain, or an op whose inputs arrived on two
    DMA queues); this walrus build refuses those.  Splitting the extras into
    single-wait NOPs directly before the instruction on the same engine is
    semantically identical (engines execute their block instructions in
    order)."""
    from concourse import mybir

    for fn in nc.m.functions:
        for bb in fn.blocks:
            out = []
            for inst in bb.instructions:
                si = inst.sync_info
                if si is not None and si.on_wait and len(si.on_wait) > 1:
                    waits = list(si.on_wait)
                    for j, w in enumerate(waits[:-1]):
                        out.append(mybir.InstNoOp(
                            name=f"{inst.name}-sw{j}", engine=inst.engine,
                            sync_info=mybir.SyncInfo(on_wait=[w], on_update=[]),
                            ins=[], outs=[]))
                    inst.sync_info = mybir.SyncInfo(
                        on_wait=[waits[-1]], on_update=list(si.on_update))
                out.append(inst)
            bb.instructions = out


C_DVB, C_DV8, C_AC8 = 0.74, 1.26, 1.95
DVB_PREFIX = 8             # pure-bf16 DVE warmup before fp8 pairs interleave
CH_AVAIL = [2.0, 3.5, 6.5, 8.0]


def _dve_order():
    """DVE emission order: a pure-bf16 warmup burst (so the PE never starves
    while ACT/fp8 production ramps), then DVB slabs with DV8 pairs injected
    at an even rate."""
    dvb = [('bf16', ch, b) for ch in range(DCH) for b in DVB_ROWS]
    dv8 = [('pair', 'dv8', cp, b) for b in DV8_ROWS for cp in range(2)]
    out = dvb[:DVB_PREFIX]
    rest = dvb[DVB_PREFIX:]
    npair = len(dv8)
    di = 0
    for k in range(npair):
        take = (len(rest) * (k + 1)) // npair - (len(rest) * k) // npair
        out.extend(rest[di : di + take])
        di += take
        out.append(dv8[k])
    out.extend(rest[di:])
    return out


def _schedule():
    """Estimated-ready-time ordering of PE work items.

    Items: ('bf16', ch, b) one bf16 slab; ('pair', cls, chpair, b) one fp8
    DoubleRow pair; ('corr',) the rank-2 correction.  Producer timelines use
    measured per-slab costs (us) and DMA chunk-availability floors."""
    items = [(4.0, 0, ('corr',))]
    seq = 1

    t = 0.0
    for it in _dve_order():
        if it[0] == 'bf16':
            t = max(t, CH_AVAIL[it[1]]) + C_DVB
        else:
            t = max(t, CH_AVAIL[2 * it[2] + 1]) + 2 * C_DV8
        items.append((t, seq, it))
        seq += 1

    t = 0.0
    for b in AC8_ROWS:
        for cp in range(2):
            t = max(t, CH_AVAIL[2 * cp + 1]) + 2 * C_AC8
            items.append((t, seq, ('pair', 'ac8', cp, b)))
            seq += 1

    items.sort(key=lambda x: (x[0], x[1]))
    return [it for _, _, it in items]


def _build_nc():
    import concourse.bass as bass
    import concourse.tile as tile
    from concourse import mybir

    f32 = mybir.dt.float32
    f32r = mybir.dt.float32r
    bf16 = mybir.dt.bfloat16
    fp8 = mybir.dt.float8e4
    nc = bass.Bass()

    s2t_d = nc.declare_dram_parameter("s2t", [D, N], bf16, isOutput=False)
    a2t_d = nc.declare_dram_parameter("a2t", [DCH, 128, BSH], f32, isOutput=False)
    # bf16 sliding-window sign tile: zero except col 31 = -sign per chunk
    sgnn_d = nc.declare_dram_parameter("sgnn", [128, DCH, 63], bf16, isOutput=False)
    # fp8 sign tiles [128, 2, DCH, 64]: [:,0]= -sign (DVE/GPS), [:,1]= +sign (ACT)
    sgn8_d = nc.declare_dram_parameter("sgn8", [128, 2, DCH, 64], fp8, isOutput=False)
    # corr[0] = clhs [2, BSH], corr[1] = crhs [2, N] combined
    corr_d = nc.declare_dram_parameter("corr", [2, BSH + N], f32r, isOutput=False)
    # labels packed [128, chunk, C] (host-padded to 2048 rows), bf16 (0/1 exact)
    lab_d = nc.declare_dram_parameter("labels", [128, NLAB, C], bf16, isOutput=False)
    ident_d = nc.declare_dram_parameter("ident", [32, 32], bf16, isOutput=False)
    recb_d = nc.declare_dram_parameter("recb", [BSH, C], f32, isOutput=False)
    out_d = nc.declare_dram_parameter("out", [BSH, C], f32, isOutput=True)

    with tile.TileContext(nc) as tc:
        with (
            tc.tile_pool(name="const", bufs=1) as const,
            tc.tile_pool(name="dslab", bufs=8) as dpool,
            tc.tile_pool(name="v8", bufs=4) as v8pool,
            tc.tile_pool(name="a8", bufs=4) as a8pool,
            tc.tile_pool(name="bank", bufs=8, space="PSUM") as bankp,
        ):
            # ---- constant loads, issued in PARALLEL across the Sync,
            # ACT and DVE sequencers (a serial stream on Sync alone costs
            # ~600-700ns per dma_start and delayed first compute to ~15us;
            # NEVER issue via gpsimd/Pool: any Q7 activity throttles every
            # other engine by ~20%).
            s2t0 = const.tile([128, N], bf16, name="s2t0", tag="s2t0")
            nc.sync.dma_start(s2t0[:], s2t_d[0:128, :])
            a2t = const.tile([128, DCH * BSH], f32, name="a2t", tag="a2t")
            nc.sync.dma_start(
                a2t[:].rearrange("p (c b) -> p c b", c=DCH),
                a2t_d[:].rearrange("c p b -> p c b"),
            )
            sgnn = const.tile([128, DCH, 63], bf16, name="sgnn", tag="sgnn")
            nc.sync.dma_start(sgnn[:], sgnn_d[:])
            s2t1 = const.tile([128, N], bf16, name="s2t1", tag="s2t1")
            nc.sync.dma_start(s2t1[:], s2t_d[128:256, :])
            corr = const.tile([2, BSH + N], f32r, name="corr", tag="corr")
            nc.sync.dma_start(corr[:], corr_d[:])
            sgn8 = const.tile([128, 2, DCH, 64], fp8, name="sgn8", tag="sgn8")
            nc.sync.dma_start(sgn8[:], sgn8_d[:])
            s2t23 = const.tile([128, 2 * N], bf16, name="s2t23", tag="s2t23")
            nc.scalar.dma_start(
                s2t23[:].rearrange("p (c n) -> p c n", c=2),
                s2t_d[:].rearrange("(c p) n -> p c n", p=128)[:, 2:4, :],
            )
            labs = const.tile([128, NLAB, C], bf16, name="labs", tag="labs")
            nc.scalar.dma_start(labs[:], lab_d[:])
            recb = const.tile([BSH, C], f32, name="recb", tag="recb")
            nc.scalar.dma_start(recb[:], recb_d[:])
            ident = const.tile([32, 32], bf16, name="ident", tag="ident")
            nc.scalar.dma_start(ident[:], ident_d[:])
            s2t = [s2t0, s2t1] + [
                s2t23[:, ch * N : (ch + 1) * N] for ch in range(2)
            ]
            clhs = corr[:, :BSH]
            crhs = corr[:, BSH:]

            psc = [
                bankp.tile([BSH, SEG], f32, name=f"psc{s}", tag="bank")
                for s in range(NSEG)
            ]

            # ---- producer emission (per-engine program order must match the
            # schedule's timeline assumptions)
            dvb_slab = {}
            pair_tiles = {}
            for it in _dve_order():
                if it[0] == 'bf16':
                    _, ch, b = it
                    slab = dpool.tile([128, N], bf16, name="dslab", tag="dslab")
                    nc.vector.tensor_scalar(
                        slab[:], s2t[ch][:],
                        a2t[:, ch * BSH + b : ch * BSH + b + 1], 0.0,
                        mybir.AluOpType.subtract, mybir.AluOpType.min,
                    )
                    dvb_slab[(ch, b)] = slab
                else:
                    _, _, cp, b = it
                    pt = v8pool.tile([128, 2, N], fp8, name="v8slab", tag="v8slab")
                    for i in range(2):
                        ch = 2 * cp + i
                        nc.vector.tensor_scalar(
                            pt[:, i, :], s2t[ch][:],
                            a2t[:, ch * BSH + b : ch * BSH + b + 1], 0.0,
                            mybir.AluOpType.subtract, mybir.AluOpType.min,
                        )
                    pair_tiles[('dv8', cp, b)] = pt
            for b in AC8_ROWS:
                for cp in range(2):
                    pt = a8pool.tile([128, 2, N], fp8, name="a8slab", tag="a8slab")
                    for i in range(2):
                        ch = 2 * cp + i
                        nc.scalar.activation(
                            pt[:, i, :], s2t[ch][:],
                            mybir.ActivationFunctionType.Relu,
                            bias=a2t[:, ch * BSH + b : ch * BSH + b + 1],
                            scale=-1.0,
                        )
                    pair_tiles[('ac8', cp, b)] = pt
            # preload the sigmoid ACT table while the PE drains (free insurance
            # against a ~1.3us table swap on the critical tail)
            sigwarm = const.tile([32, 1], bf16, name="sigwarm", tag="sigwarm")
            nc.scalar.activation(
                sigwarm[:], ident[:, 0:1],
                mybir.ActivationFunctionType.Sigmoid,
            )
            # ---- PE stream in estimated-ready order
            items = _schedule()
            started = False
            for idx, it in enumerate(items):
                last = idx == len(items) - 1
                start = not started
                started = True
                if it[0] == 'corr':
                    for s in range(NSEG):
                        nc.tensor.matmul(
                            psc[s][:], clhs,
                            crhs[:, SEG * s : SEG * (s + 1)],
                            start=start, stop=last, skip_group_check=True,
                        )
                elif it[0] == 'bf16':
                    _, ch, b = it
                    slab = dvb_slab[(ch, b)]
                    lhs = sgnn[:, ch, 31 - b : 63 - b]
                    for s in range(NSEG):
                        nc.tensor.matmul(
                            psc[s][:], lhs,
                            slab[:, SEG * s : SEG * (s + 1)],
                            start=start, stop=last, skip_group_check=True,
                        )
                else:
                    _, cls, cp, b = it
                    pt = pair_tiles[(cls, cp, b)]
                    k = 1 if cls == 'ac8' else 0
                    lhs = sgn8[:, k, 2 * cp : 2 * cp + 2, 31 - b : 63 - b]
                    for s in range(NSEG):
                        nc.tensor.matmul(
                            psc[s][:], lhs,
                            pt[:, :, SEG * s : SEG * (s + 1)],
                            start=start, stop=last, skip_group_check=True,
                            perf_mode=mybir.MatmulPerfMode.DoubleRow,
                        )

            # ---- sigmoid (PSUM -> SBUF, bf16) ----
            ssig = const.tile([BSH, N], bf16, name="ssig", tag="ssig")
            for s in range(NSEG):
                nc.scalar.activation(
                    ssig[:, SEG * s : SEG * (s + 1)], psc[s][:],
                    mybir.ActivationFunctionType.Sigmoid,
                )

            # ---- 16 transposes into ONE PSUM bank, one copy, final matmuls
            tpall = bankp.tile([128, NLAB * BSH], bf16, name="tpall", tag="bank")
            for k in range(NLAB):
                pk = min(128, N - 128 * k)
                nc.tensor.transpose(
                    tpall[:pk, BSH * k : BSH * k + BSH],
                    ssig[:, 128 * k : 128 * k + pk], ident[:],
                )
            sct = const.tile([128, NLAB * BSH], bf16, name="sct", tag="sct")
            nc.vector.tensor_copy(sct[:], tpall[:])
            out_ps = bankp.tile([BSH, C], f32, name="out_ps", tag="bank")
            for k in range(NLAB):
                pk = min(128, N - 128 * k)
                nc.tensor.matmul(
                    out_ps[:], sct[:pk, BSH * k : BSH * k + BSH],
                    labs[:pk, k, :],
                    start=(k == 0), stop=(k == NLAB - 1),
                )

            # ---- divide by counts, write out ----
            out_s = const.tile([BSH, C], f32, name="out_s", tag="out_s")
            nc.vector.tensor_mul(out_s[:], out_ps[:], recb[:])
            nc.sync.dma_start(out_d[:], out_s[:])

    _split_multi_waits(nc)
    return nc


def _prep_host(inputs, support_tensors, support_labels, kernel_w, kernel_b):
    import ml_dtypes

    bf16 = ml_dtypes.bfloat16
    e4m3 = ml_dtypes.float8_e4m3
    a = np.asarray(inputs, dtype=np.float32)
    S = np.asarray(support_tensors, dtype=np.float32)
    L = np.asarray(support_labels, dtype=np.float32)
    w = np.asarray(kernel_w, dtype=np.float32)
    kb = np.float32(np.asarray(kernel_b, dtype=np.float32))

    aw = 2.0 * np.abs(w)
    sgn = np.sign(w).astype(np.float32)
    s2t = np.ascontiguousarray((S * aw[None, :]).T).astype(bf16)   # [D, N]
    wS = (S @ w).astype(np.float32)                                # [N]
    wa = (a @ w).astype(np.float32)                                # [B]
    a2 = a * aw[None, :]                                           # [B, D]

    # sliding-window sign tiles: col 31 = -/+ sign chunk
    sgn_chunks = sgn.reshape(DCH, 128).T                           # [128, DCH]
    sgnn = np.zeros((128, DCH, 63), dtype=np.float32)
    sgnn[:, :, 31] = -sgn_chunks
    sgn8 = np.zeros((128, 2, DCH, 64), dtype=np.float32)
    sgn8[:, 0, :, 31] = -sgn_chunks
    sgn8[:, 1, :, 31] = sgn_chunks
    labp = np.zeros((NP, C), dtype=np.float32)
    labp[:N] = L
    labp = np.ascontiguousarray(
        labp.reshape(NLAB, 128, C).transpose(1, 0, 2)).astype(bf16)
    ident = np.eye(32, dtype=bf16)
    counts = L.sum(axis=0)
    recip = np.where(counts != 0, 1.0 / np.maximum(counts, 1e-30), 0.0)
    recb = np.broadcast_to(recip.astype(np.float32), (BSH, C)).copy()

    shared = {
        "s2t": s2t, "sgnn": sgnn.astype(bf16), "sgn8": sgn8.astype(e4m3),
        "labels": labp, "ident": ident, "recb": recb,
    }
    in_maps = []
    for c in range(NCORES):
        rows = slice(BSH * c, BSH * (c + 1))
        a2t_c = np.ascontiguousarray(
            a2[rows].T.reshape(DCH, 128, BSH))                     # [DCH,128,BSH]
        corr_c = np.empty((2, BSH + N), dtype=np.float32)
        corr_c[0, :BSH] = kb - wa[rows]
        corr_c[1, :BSH] = 1.0
        corr_c[0, BSH:] = 1.0
        corr_c[1, BSH:] = wS
        in_maps.append(dict(shared, a2t=a2t_c, corr=corr_c))
    return in_maps


def kernel(**inputs) -> np.ndarray:
    from concourse.bass_utils import run_bass_kernel_spmd

    if "nc" not in _CACHE:
        _CACHE["nc"] = _build_nc()
    nc = _CACHE["nc"]

    in_maps = _prep_host(
        inputs["inputs"], inputs["support_tensors"], inputs["support_labels"],
        inputs["kernel_w"], inputs["kernel_b"],
    )
    res = run_bass_kernel_spmd(nc, in_maps, list(range(NCORES)))
    return np.concatenate([res.results[i]["out"] for i in range(NCORES)], axis=0)


# revision 9
# speedup vs baseline: 1.2207x; 1.0341x over previous
"""Trainium2 Bass kernel for the siamese-kNN classification head.

Reference computation (B=256, N=2000, D=512, C=100):
    scores[b,n] = sigmoid(sum_d w_d * |a[b,d] - S[n,d]| + kb)
    out[b,c]    = (scores @ L)[b,c] / count_c     (0 where count_c == 0)

Strategy
--------
Data-parallel over the batch: core i handles rows 32*i .. 32*i+32, no
collectives.  The pairwise |a-s| volume (32*2000*512 element-touches per
core) dominates; |x| = relu(2x) - x splits each score into a nonlinear
slab relu(A''-S'') (A''=2|w|a, S''=2|w|S) that one engine produces in a
single instruction per [128,2000] d-chunk, plus a separable linear part
folded into a rank-2 f32r correction matmul.  The PE reduces each slab
over d with a sliding-window +-sign stationary (col b of a [128,32]
window = sign, rest zero).

The baseline was PE-ingest-bound (1 bf16 column/cycle -> ~110us/core).
This version converts most slabs to fp8e4 and reduces PAIRS of them in
one DoubleRow matmul (stationary [128,2,32], moving [128,2,seg], 0.5
cycles/column -> 2x PE throughput).  Slab production is spread over
two engines, batch rows assigned per class:
  - DVB rows: DVE tensor_scalar bf16 slabs (2x DVE mode), bf16 matmul.
  - DV8 rows: DVE fp8 slabs (1x mode), DoubleRow pairs (chunks 01/23).
  - AC8 rows: ACT activation(Relu) fp8 slabs, DoubleRow pairs.
The PE instruction stream is ordered by estimated slab-ready times so
no engine stalls the pipe.  fp8e4 quantization of the slab keeps max
rel err ~1.5e-2 (< 2e-2 gate); DVB rows stay bf16-accurate.

DoubleRow ISA restrictions honored: sign-window planes have stride 64
(even, 16B-aligned), moving-plane stride N=2000, all fp8 offsets even,
PSUM dst starts at partition 0.
"""

import sys

for _p in ("/opt/trn_rl_repo", "/root/.axon_site/_ro/trn_rl_repo"):
    if _p not in sys.path:
        sys.path.append(_p)

import numpy as np

B, N, D, C = 256, 2000, 512, 100
NP = 2048                  # label rows padded to 16 full chunks
NCORES = 8
BSH = B // NCORES          # 32 batch rows per core
DCH = D // 128             # 4 d-chunks
NSEG = 4                   # PSUM free-dim segments
SEG = N // NSEG            # 500
NLAB = NP // 128           # 16 label chunks

# batch-row classes (per core): counts must sum to BSH
# (GPSIMD is NOT used: its tensor_scalar runs ~15ns/elem on HW and poisons
# concurrent DVE/ACT throughput via SBUF contention)
N_DVB = 11                 # DVE bf16 rows
N_DV8 = 10                 # DVE fp8 rows
N_AC8 = 11                 # ACT fp8 rows
assert N_DVB + N_DV8 + N_AC8 == BSH

DVB_ROWS = list(range(0, N_DVB))
DV8_ROWS = list(range(N_DVB, N_DVB + N_DV8))
AC8_ROWS = list(range(N_DVB + N_DV8, BSH))

_CACHE = {}


def _split_multi_waits(nc):
    """TRN2 TPB instructions encode at most ONE semaphore wait, but Tile can
    attach several (e.g. the tail drain, or an op whose inputs arrived on two
    DMA queues); this walrus build refuses those.  Splitting the extras into
    single-wait NOPs directly before the instruction on the same engine is
    semantically identical (engines execute their block instructions in
    order)."""
    from concourse import mybir

    for fn in nc.m.functions:
        for bb in fn.blocks:
            out = []
            for inst in bb.instructions:
                si = inst.sync_info
                if si is not None and si.on_wait and len(si.on_wait) > 1:
                    waits = list(si.on_wait)
                    for j, w in enumerate(waits[:-1]):
                        out.append(mybir.InstNoOp(
                            name=f"{inst.name}-sw{j}", engine=inst.engine,
                            sync_info=mybir.SyncInfo(on_wait=[w], on_update=[]),
                            ins=[], outs=[]))
                    inst.sync_info = mybir.SyncInfo(
                        on_wait=[waits[-1]], on_update=list(si.on_update))
                out.append(inst)
            bb.instructions = out


CH_AVAIL = [13.2, 14.6, 16.1, 17.6]
C_DVB, C_DV8, C_AC8 = 0.74, 1.26, 1.95
DVB_PREFIX = 8             # pure-bf16 DVE warmup before fp8 pairs interleave
N_WARM = 14                # PE pstate-warmup matmuls on scratch
CH_AVAIL = [2.0, 3.5, 6.5, 8.0]


def _dve_order():
    """DVE emission order: a pure-bf16 warmup burst (so the PE never starves
    while ACT/fp8 production ramps), then DVB slabs with DV8 pairs injected
    at an even rate."""
    dvb = [('bf16', ch, b) for ch in range(DCH) for b in DVB_ROWS]
    dv8 = [('pair', 'dv8', cp, b) for b in DV8_ROWS for cp in range(2)]
    out = dvb[:DVB_PREFIX]
    rest = dvb[DVB_PREFIX:]
    npair = len(dv8)
    di = 0
    for k in range(npair):
        take = (len(rest) * (k + 1)) // npair - (len(rest) * k) // npair
        out.extend(rest[di : di + take])
        di += take
        out.append(dv8[k])
    out.extend(rest[di:])
    return out


def _schedule():
    """Estimated-ready-time ordering of PE work items.

    Items: ('bf16', ch, b) one bf16 slab; ('pair', cls, chpair, b) one fp8
    DoubleRow pair; ('corr',) the rank-2 correction.  Producer timelines use
    measured per-slab costs (us) and DMA chunk-availability floors."""
    items = [(9.7, 0, ('corr',))]
    seq = 1

    t = 13.2
    for it in _dve_order():
        if it[0] == 'bf16':
            t = max(t, CH_AVAIL[it[1]]) + C_DVB
        else:
            t = max(t, CH_AVAIL[2 * it[2] + 1]) + 2 * C_DV8
        items.append((t, seq, it))
        seq += 1

    t = 13.2
    for b in AC8_ROWS:
        for cp in range(2):
            t = max(t, CH_AVAIL[2 * cp + 1]) + 2 * C_AC8
            items.append((t, seq, ('pair', 'ac8', cp, b)))
            seq += 1

    items.sort(key=lambda x: (x[0], x[1]))
    return [it for _, _, it in items]


def _build_nc():
    import concourse.bass as bass
    import concourse.tile as tile
    from concourse import mybir

    f32 = mybir.dt.float32
    f32r = mybir.dt.float32r
    bf16 = mybir.dt.bfloat16
    fp8 = mybir.dt.float8e4
    nc = bass.Bass()

    s2t_d = nc.declare_dram_parameter("s2t", [D, N], bf16, isOutput=False)
    a2t_d = nc.declare_dram_parameter("a2t", [DCH, 128, BSH], f32, isOutput=False)
    # bf16 sliding-window sign tile: zero except col 31 = -sign per chunk
    sgnn_d = nc.declare_dram_parameter("sgnn", [128, DCH, 63], bf16, isOutput=False)
    # fp8 sign tiles [128, 2, DCH, 64]: [:,0]= -sign (DVE/GPS), [:,1]= +sign (ACT)
    sgn8_d = nc.declare_dram_parameter("sgn8", [128, 2, DCH, 64], fp8, isOutput=False)
    # corr[0] = clhs [2, BSH], corr[1] = crhs [2, N] combined
    corr_d = nc.declare_dram_parameter("corr", [2, BSH + N], f32r, isOutput=False)
    # labels packed [128, chunk, C] (host-padded to 2048 rows), bf16 (0/1 exact)
    lab_d = nc.declare_dram_parameter("labels", [128, NLAB, C], bf16, isOutput=False)
    ident_d = nc.declare_dram_parameter("ident", [32, 32], bf16, isOutput=False)
    recb_d = nc.declare_dram_parameter("recb", [BSH, C], f32, isOutput=False)
    out_d = nc.declare_dram_parameter("out", [BSH, C], f32, isOutput=True)

    with tile.TileContext(nc) as tc:
        with (
            tc.tile_pool(name="const", bufs=1) as const,
            tc.tile_pool(name="dslab", bufs=8) as dpool,
            tc.tile_pool(name="v8", bufs=4) as v8pool,
            tc.tile_pool(name="a8", bufs=4) as a8pool,
            tc.tile_pool(name="bank", bufs=8, space="PSUM") as bankp,
        ):
            # ---- constant loads, issued in PARALLEL across the Sync and
            # ACT sequencers (a serial stream on Sync alone costs ~600-700ns
            # per dma_start and delayed first compute to ~15us; NEVER issue
            # via gpsimd/Pool: any Q7 activity throttles every other engine
            # by ~20%).  Sync's hw queue streams the four 512KB s2t chunks;
            # the ACT queue lands every small critical constant by ~9us.
            s2t0 = const.tile([128, N], bf16, name="s2t0", tag="s2t0")
            nc.sync.dma_start(s2t0[:], s2t_d[0:128, :])
            s2t1 = const.tile([128, N], bf16, name="s2t1", tag="s2t1")
            nc.sync.dma_start(s2t1[:], s2t_d[128:256, :])
            s2t23 = const.tile([128, 2 * N], bf16, name="s2t23", tag="s2t23")
            nc.sync.dma_start(
                s2t23[:].rearrange("p (c n) -> p c n", c=2),
                s2t_d[:].rearrange("(c p) n -> p c n", p=128)[:, 2:4, :],
            )
            labs = const.tile([128, NLAB, C], bf16, name="labs", tag="labs")
            nc.sync.dma_start(labs[:], lab_d[:])
            recb = const.tile([BSH, C], f32, name="recb", tag="recb")
            nc.sync.dma_start(recb[:], recb_d[:])
            ident = const.tile([32, 32], bf16, name="ident", tag="ident")
            nc.sync.dma_start(ident[:], ident_d[:])
            a2t = const.tile([128, DCH * BSH], f32, name="a2t", tag="a2t")
            nc.scalar.dma_start(
                a2t[:].rearrange("p (c b) -> p c b", c=DCH),
                a2t_d[:].rearrange("c p b -> p c b"),
            )
            sgnn = const.tile([128, DCH, 63], bf16, name="sgnn", tag="sgnn")
            nc.scalar.dma_start(sgnn[:], sgnn_d[:])
            corr = const.tile([2, BSH + N], f32r, name="corr", tag="corr")
            nc.scalar.dma_start(corr[:], corr_d[:])
            sgn8 = const.tile([128, 2, DCH, 64], fp8, name="sgn8", tag="sgn8")
            nc.scalar.dma_start(sgn8[:], sgn8_d[:])
            s2t = [s2t0, s2t1] + [
                s2t23[:, ch * N : (ch + 1) * N] for ch in range(2)
            ]
            clhs = corr[:, :BSH]
            crhs = corr[:, BSH:]

            psc = [
                bankp.tile([BSH, SEG], f32, name=f"psc{s}", tag="bank")
                for s in range(NSEG)
            ]

            # ---- producer emission (per-engine program order must match the
            # schedule's timeline assumptions)
            dvb_slab = {}
            pair_tiles = {}
            for it in _dve_order():
                if it[0] == 'bf16':
                    _, ch, b = it
                    slab = dpool.tile([128, N], bf16, name="dslab", tag="dslab")
                    nc.vector.tensor_scalar(
                        slab[:], s2t[ch][:],
                        a2t[:, ch * BSH + b : ch * BSH + b + 1], 0.0,
                        mybir.AluOpType.subtract, mybir.AluOpType.min,
                    )
                    dvb_slab[(ch, b)] = slab
                else:
                    _, _, cp, b = it
                    pt = v8pool.tile([128, 2, N], fp8, name="v8slab", tag="v8slab")
                    for i in range(2):
                        ch = 2 * cp + i
                        nc.vector.tensor_scalar(
                            pt[:, i, :], s2t[ch][:],
                            a2t[:, ch * BSH + b : ch * BSH + b + 1], 0.0,
                            mybir.AluOpType.subtract, mybir.AluOpType.min,
                        )
                    pair_tiles[('dv8', cp, b)] = pt
            for b in AC8_ROWS:
                for cp in range(2):
                    pt = a8pool.tile([128, 2, N], fp8, name="a8slab", tag="a8slab")
                    for i in range(2):
                        ch = 2 * cp + i
                        nc.scalar.activation(
                            pt[:, i, :], s2t[ch][:],
                            mybir.ActivationFunctionType.Relu,
                            bias=a2t[:, ch * BSH + b : ch * BSH + b + 1],
                            scale=-1.0,
                        )
                    pair_tiles[('ac8', cp, b)] = pt
            # preload the sigmoid ACT table while the PE drains (free insurance
            # against a ~1.3us table swap on the critical tail)
            sigwarm = const.tile([32, 1], bf16, name="sigwarm", tag="sigwarm")
            nc.scalar.activation(
                sigwarm[:], ident[:, 0:1],
                mybir.ActivationFunctionType.Sigmoid,
            )
            # ---- PE pstate warmup: harmless matmuls on a zeroed scratch
            # tile while the s2t DMAs stream in, so the PE clock is ramped
            # when real work arrives
            scratch = const.tile([128, 512], bf16, name="scratch", tag="scratch")
            nc.vector.memset(scratch[:], 0)
            ps_warm = bankp.tile([32, 512], f32, name="ps_warm", tag="bank")
            for _ in range(N_WARM):
                nc.tensor.matmul(
                    ps_warm[:], scratch[:, 0:32], scratch[:],
                    start=True, stop=True, skip_group_check=True,
                )

            # ---- PE stream in estimated-ready order
            items = _schedule()
            started = False
            for idx, it in enumerate(items):
                last = idx == len(items) - 1
                start = not started
                started = True
                if it[0] == 'corr':
                    for s in range(NSEG):
                        nc.tensor.matmul(
                            psc[s][:], clhs,
                            crhs[:, SEG * s : SEG * (s + 1)],
                            start=start, stop=last, skip_group_check=True,
                        )
                elif it[0] == 'bf16':
                    _, ch, b = it
                    slab = dvb_slab[(ch, b)]
                    lhs = sgnn[:, ch, 31 - b : 63 - b]
                    for s in range(NSEG):
                        nc.tensor.matmul(
                            psc[s][:], lhs,
                            slab[:, SEG * s : SEG * (s + 1)],
                            start=start, stop=last, skip_group_check=True,
                        )
                else:
                    _, cls, cp, b = it
                    pt = pair_tiles[(cls, cp, b)]
                    k = 1 if cls == 'ac8' else 0
                    lhs = sgn8[:, k, 2 * cp : 2 * cp + 2, 31 - b : 63 - b]
                    for s in range(NSEG):
                        nc.tensor.matmul(
                            psc[s][:], lhs,
                            pt[:, :, SEG * s : SEG * (s + 1)],
                            start=start, stop=last, skip_group_check=True,
                            perf_mode=mybir.MatmulPerfMode.DoubleRow,
                        )

            # ---- sigmoid (PSUM -> SBUF, bf16) ----
            ssig = const.tile([BSH, N], bf16, name="ssig", tag="ssig")
            for s in range(NSEG):
                nc.scalar.activation(
                    ssig[:, SEG * s : SEG * (s + 1)], psc[s][:],
                    mybir.ActivationFunctionType.Sigmoid,
                )

            # ---- 16 transposes into ONE PSUM bank, one copy, final matmuls
            tpall = bankp.tile([128, NLAB * BSH], bf16, name="tpall", tag="bank")
            for k in range(NLAB):
                pk = min(128, N - 128 * k)
                nc.tensor.transpose(
                    tpall[:pk, BSH * k : BSH * k + BSH],
                    ssig[:, 128 * k : 128 * k + pk], ident[:],
                )
            sct = const.tile([128, NLAB * BSH], bf16, name="sct", tag="sct")
            nc.vector.tensor_copy(sct[:], tpall[:])
            out_ps = bankp.tile([BSH, C], f32, name="out_ps", tag="bank")
            for k in range(NLAB):
                pk = min(128, N - 128 * k)
                nc.tensor.matmul(
                    out_ps[:], sct[:pk, BSH * k : BSH * k + BSH],
                    labs[:pk, k, :],
                    start=(k == 0), stop=(k == NLAB - 1),
                )

            # ---- divide by counts, write out ----
            out_s = const.tile([BSH, C], f32, name="out_s", tag="out_s")
            nc.vector.tensor_mul(out_s[:], out_ps[:], recb[:])
            nc.sync.dma_start(out_d[:], out_s[:])

    _split_multi_waits(nc)
    return nc


def _prep_host(inputs, support_tensors, support_labels, kernel_w, kernel_b):
    import ml_dtypes

    bf16 = ml_dtypes.bfloat16
    e4m3 = ml_dtypes.float8_e4m3
    a = np.asarray(inputs, dtype=np.float32)
    S = np.asarray(support_tensors, dtype=np.float32)
    L = np.asarray(support_labels, dtype=np.float32)
    w = np.asarray(kernel_w, dtype=np.float32)
    kb = np.float32(np.asarray(kernel_b, dtype=np.float32))

    aw = 2.0 * np.abs(w)
    sgn = np.sign(w).astype(np.float32)
    s2t = np.ascontiguousarray((S * aw[None, :]).T).astype(bf16)   # [D, N]
    wS = (S @ w).astype(np.float32)                                # [N]
    wa = (a @ w).astype(np.float32)                                # [B]
    a2 = a * aw[None, :]                                           # [B, D]

    # sliding-window sign tiles: col 31 = -/+ sign chunk
    sgn_chunks = sgn.reshape(DCH, 128).T                           # [128, DCH]
    sgnn = np.zeros((128, DCH, 63), dtype=np.float32)
    sgnn[:, :, 31] = -sgn_chunks
    sgn8 = np.zeros((128, 2, DCH, 64), dtype=np.float32)
    sgn8[:, 0, :, 31] = -sgn_chunks
    sgn8[:, 1, :, 31] = sgn_chunks
    labp = np.zeros((NP, C), dtype=np.float32)
    labp[:N] = L
    labp = np.ascontiguousarray(
        labp.reshape(NLAB, 128, C).transpose(1, 0, 2)).astype(bf16)
    ident = np.eye(32, dtype=bf16)
    counts = L.sum(axis=0)
    recip = np.where(counts != 0, 1.0 / np.maximum(counts, 1e-30), 0.0)
    recb = np.broadcast_to(recip.astype(np.float32), (BSH, C)).copy()

    shared = {
        "s2t": s2t, "sgnn": sgnn.astype(bf16), "sgn8": sgn8.astype(e4m3),
        "labels": labp, "ident": ident, "recb": recb,
    }
    in_maps = []
    for c in range(NCORES):
        rows = slice(BSH * c, BSH * (c + 1))
        a2t_c = np.ascontiguousarray(
            a2[rows].T.reshape(DCH, 128, BSH))                     # [DCH,128,BSH]
        corr_c = np.empty((2, BSH + N), dtype=np.float32)
        corr_c[0, :BSH] = kb - wa[rows]
        corr_c[1, :BSH] = 1.0
        corr_c[0, BSH:] = 1.0
        corr_c[1, BSH:] = wS
        in_maps.append(dict(shared, a2t=a2t_c, corr=corr_c))
    return in_maps


def kernel(**inputs) -> np.ndarray:
    from concourse.bass_utils import run_bass_kernel_spmd

    if "nc" not in _CACHE:
        _CACHE["nc"] = _build_nc()
    nc = _CACHE["nc"]

    in_maps = _prep_host(
        inputs["inputs"], inputs["support_tensors"], inputs["support_labels"],
        inputs["kernel_w"], inputs["kernel_b"],
    )
    res = run_bass_kernel_spmd(nc, in_maps, list(range(NCORES)))
    return np.concatenate([res.results[i]["out"] for i in range(NCORES)], axis=0)


# revision 10
# speedup vs baseline: 1.2391x; 1.0151x over previous
"""Trainium2 Bass kernel for the siamese-kNN classification head.

Reference computation (B=256, N=2000, D=512, C=100):
    scores[b,n] = sigmoid(sum_d w_d * |a[b,d] - S[n,d]| + kb)
    out[b,c]    = (scores @ L)[b,c] / count_c     (0 where count_c == 0)

Strategy
--------
Data-parallel over the batch: core i handles rows 32*i .. 32*i+32, no
collectives.  The pairwise |a-s| volume (32*2000*512 element-touches per
core) dominates; |x| = relu(2x) - x splits each score into a nonlinear
slab relu(A''-S'') (A''=2|w|a, S''=2|w|S) that one engine produces in a
single instruction per [128,2000] d-chunk, plus a separable linear part
folded into a rank-2 f32r correction matmul.  The PE reduces each slab
over d with a sliding-window +-sign stationary (col b of a [128,32]
window = sign, rest zero).

The baseline was PE-ingest-bound (1 bf16 column/cycle -> ~110us/core).
This version converts most slabs to fp8e4 and reduces PAIRS of them in
one DoubleRow matmul (stationary [128,2,32], moving [128,2,seg], 0.5
cycles/column -> 2x PE throughput).  Slab production is spread over
two engines, batch rows assigned per class:
  - DVB rows: DVE tensor_scalar bf16 slabs (2x DVE mode), bf16 matmul.
  - DV8 rows: DVE fp8 slabs (1x mode), DoubleRow pairs (chunks 01/23).
  - AC8 rows: ACT activation(Relu) fp8 slabs, DoubleRow pairs.
The PE instruction stream is ordered by estimated slab-ready times so
no engine stalls the pipe.  fp8e4 quantization of the slab keeps max
rel err ~1.5e-2 (< 2e-2 gate); DVB rows stay bf16-accurate.

DoubleRow ISA restrictions honored: sign-window planes have stride 64
(even, 16B-aligned), moving-plane stride N=2000, all fp8 offsets even,
PSUM dst starts at partition 0.
"""

import sys

for _p in ("/opt/trn_rl_repo", "/root/.axon_site/_ro/trn_rl_repo"):
    if _p not in sys.path:
        sys.path.append(_p)

import numpy as np

B, N, D, C = 256, 2000, 512, 100
NP = 2048                  # label rows padded to 16 full chunks
NCORES = 8
BSH = B // NCORES          # 32 batch rows per core
DCH = D // 128             # 4 d-chunks
NSEG = 4                   # PSUM free-dim segments
SEG = N // NSEG            # 500
NLAB = NP // 128           # 16 label chunks

# batch-row classes (per core): counts must sum to BSH
# (GPSIMD is NOT used: its tensor_scalar runs ~15ns/elem on HW and poisons
# concurrent DVE/ACT throughput via SBUF contention)
N_DVB = 11                 # DVE bf16 rows
N_DV8 = 11                 # DVE fp8 rows
N_AC8 = 10                 # ACT fp8 rows
assert N_DVB + N_DV8 + N_AC8 == BSH

DVB_ROWS = list(range(0, N_DVB))
DV8_ROWS = list(range(N_DVB, N_DVB + N_DV8))
AC8_ROWS = list(range(N_DVB + N_DV8, BSH))

_CACHE = {}


def _split_multi_waits(nc):
    """TRN2 TPB instructions encode at most ONE semaphore wait, but Tile can
    attach several (e.g. the tail drain, or an op whose inputs arrived on two
    DMA queues); this walrus build refuses those.  Splitting the extras into
    single-wait NOPs directly before the instruction on the same engine is
    semantically identical (engines execute their block instructions in
    order)."""
    from concourse import mybir

    for fn in nc.m.functions:
        for bb in fn.blocks:
            out = []
            for inst in bb.instructions:
                si = inst.sync_info
                if si is not None and si.on_wait and len(si.on_wait) > 1:
                    waits = list(si.on_wait)
                    for j, w in enumerate(waits[:-1]):
                        out.append(mybir.InstNoOp(
                            name=f"{inst.name}-sw{j}", engine=inst.engine,
                            sync_info=mybir.SyncInfo(on_wait=[w], on_update=[]),
                            ins=[], outs=[]))
                    inst.sync_info = mybir.SyncInfo(
                        on_wait=[waits[-1]], on_update=list(si.on_update))
                out.append(inst)
            bb.instructions = out


CH_AVAIL = [13.2, 14.6, 16.1, 17.6]
C_DVB, C_DV8, C_AC8 = 0.74, 1.26, 1.95
DVB_PREFIX = 8             # pure-bf16 DVE warmup before fp8 pairs interleave
N_WARM = 14                # PE pstate-warmup matmuls on scratch
CH_AVAIL = [2.0, 3.5, 6.5, 8.0]


def _dve_order():
    """DVE emission order: a pure-bf16 warmup burst (so the PE never starves
    while ACT/fp8 production ramps), then DVB slabs with DV8 pairs injected
    at an even rate."""
    dvb = [('bf16', ch, b) for ch in range(DCH) for b in DVB_ROWS]
    dv8 = [('pair', 'dv8', cp, b) for b in DV8_ROWS for cp in range(2)]
    out = dvb[:DVB_PREFIX]
    rest = dvb[DVB_PREFIX:]
    npair = len(dv8)
    di = 0
    for k in range(npair):
        take = (len(rest) * (k + 1)) // npair - (len(rest) * k) // npair
        out.extend(rest[di : di + take])
        di += take
        out.append(dv8[k])
    out.extend(rest[di:])
    return out


def _schedule():
    """Estimated-ready-time ordering of PE work items.

    Items: ('bf16', ch, b) one bf16 slab; ('pair', cls, chpair, b) one fp8
    DoubleRow pair; ('corr',) the rank-2 correction.  Producer timelines use
    measured per-slab costs (us) and DMA chunk-availability floors."""
    items = [(9.7, 0, ('corr',))]
    seq = 1

    t = 13.2
    for it in _dve_order():
        if it[0] == 'bf16':
            t = max(t, CH_AVAIL[it[1]]) + C_DVB
        else:
            t = max(t, CH_AVAIL[2 * it[2] + 1]) + 2 * C_DV8
        items.append((t, seq, it))
        seq += 1

    t = 13.2
    for b in AC8_ROWS:
        for cp in range(2):
            t = max(t, CH_AVAIL[2 * cp + 1]) + 2 * C_AC8
            items.append((t, seq, ('pair', 'ac8', cp, b)))
            seq += 1

    items.sort(key=lambda x: (x[0], x[1]))
    return [it for _, _, it in items]


def _build_nc():
    import concourse.bass as bass
    import concourse.tile as tile
    from concourse import mybir

    f32 = mybir.dt.float32
    f32r = mybir.dt.float32r
    bf16 = mybir.dt.bfloat16
    fp8 = mybir.dt.float8e4
    nc = bass.Bass()

    s2t_d = nc.declare_dram_parameter("s2t", [D, N], bf16, isOutput=False)
    a2t_d = nc.declare_dram_parameter("a2t", [DCH, 128, BSH], f32, isOutput=False)
    # bf16 sliding-window sign tile: zero except col 31 = -sign per chunk
    sgnn_d = nc.declare_dram_parameter("sgnn", [128, DCH, 63], bf16, isOutput=False)
    # fp8 sign tiles [128, 2, DCH, 64]: [:,0]= -sign (DVE/GPS), [:,1]= +sign (ACT)
    sgn8_d = nc.declare_dram_parameter("sgn8", [128, 2, DCH, 64], fp8, isOutput=False)
    # corr[0] = clhs [2, BSH], corr[1] = crhs [2, N] combined
    corr_d = nc.declare_dram_parameter("corr", [2, BSH + N], f32r, isOutput=False)
    # labels packed [128, chunk, C] (host-padded to 2048 rows), bf16 (0/1 exact)
    lab_d = nc.declare_dram_parameter("labels", [128, NLAB, C], bf16, isOutput=False)
    ident_d = nc.declare_dram_parameter("ident", [32, 32], bf16, isOutput=False)
    recb_d = nc.declare_dram_parameter("recb", [BSH, C], f32, isOutput=False)
    out_d = nc.declare_dram_parameter("out", [BSH, C], f32, isOutput=True)

    with tile.TileContext(nc) as tc:
        with (
            tc.tile_pool(name="const", bufs=1) as const,
            tc.tile_pool(name="dslab", bufs=8) as dpool,
            tc.tile_pool(name="v8", bufs=4) as v8pool,
            tc.tile_pool(name="a8", bufs=4) as a8pool,
            tc.tile_pool(name="bank", bufs=8, space="PSUM") as bankp,
        ):
            # ---- constant loads, issued in PARALLEL across the Sync and
            # ACT sequencers (a serial stream on Sync alone costs ~600-700ns
            # per dma_start and delayed first compute to ~15us; NEVER issue
            # via gpsimd/Pool: any Q7 activity throttles every other engine
            # by ~20%).  Sync's hw queue streams the four 512KB s2t chunks;
            # the ACT queue lands every small critical constant by ~9us.
            s2t0 = const.tile([128, N], bf16, name="s2t0", tag="s2t0")
            nc.sync.dma_start(s2t0[:], s2t_d[0:128, :])
            s2t1 = const.tile([128, N], bf16, name="s2t1", tag="s2t1")
            nc.sync.dma_start(s2t1[:], s2t_d[128:256, :])
            s2t23 = const.tile([128, 2 * N], bf16, name="s2t23", tag="s2t23")
            nc.sync.dma_start(
                s2t23[:].rearrange("p (c n) -> p c n", c=2),
                s2t_d[:].rearrange("(c p) n -> p c n", p=128)[:, 2:4, :],
            )
            labs = const.tile([128, NLAB, C], bf16, name="labs", tag="labs")
            nc.sync.dma_start(labs[:], lab_d[:])
            recb = const.tile([BSH, C], f32, name="recb", tag="recb")
            nc.sync.dma_start(recb[:], recb_d[:])
            ident = const.tile([32, 32], bf16, name="ident", tag="ident")
            nc.sync.dma_start(ident[:], ident_d[:])
            a2t = const.tile([128, DCH * BSH], f32, name="a2t", tag="a2t")
            nc.scalar.dma_start(
                a2t[:].rearrange("p (c b) -> p c b", c=DCH),
                a2t_d[:].rearrange("c p b -> p c b"),
            )
            sgnn = const.tile([128, DCH, 63], bf16, name="sgnn", tag="sgnn")
            nc.scalar.dma_start(sgnn[:], sgnn_d[:])
            corr = const.tile([2, BSH + N], f32r, name="corr", tag="corr")
            nc.scalar.dma_start(corr[:], corr_d[:])
            sgn8 = const.tile([128, 2, DCH, 64], fp8, name="sgn8", tag="sgn8")
            nc.scalar.dma_start(sgn8[:], sgn8_d[:])
            s2t = [s2t0, s2t1] + [
                s2t23[:, ch * N : (ch + 1) * N] for ch in range(2)
            ]
            clhs = corr[:, :BSH]
            crhs = corr[:, BSH:]

            psc = [
                bankp.tile([BSH, SEG], f32, name=f"psc{s}", tag="bank")
                for s in range(NSEG)
            ]

            # ---- producer emission (per-engine program order must match the
            # schedule's timeline assumptions)
            dvb_slab = {}
            pair_tiles = {}
            for it in _dve_order():
                if it[0] == 'bf16':
                    _, ch, b = it
                    slab = dpool.tile([128, N], bf16, name="dslab", tag="dslab")
                    nc.vector.tensor_scalar(
                        slab[:], s2t[ch][:],
                        a2t[:, ch * BSH + b : ch * BSH + b + 1], 0.0,
                        mybir.AluOpType.subtract, mybir.AluOpType.min,
                    )
                    dvb_slab[(ch, b)] = slab
                else:
                    _, _, cp, b = it
                    pt = v8pool.tile([128, 2, N], fp8, name="v8slab", tag="v8slab")
                    for i in range(2):
                        ch = 2 * cp + i
                        nc.vector.tensor_scalar(
                            pt[:, i, :], s2t[ch][:],
                            a2t[:, ch * BSH + b : ch * BSH + b + 1], 0.0,
                            mybir.AluOpType.subtract, mybir.AluOpType.min,
                        )
                    pair_tiles[('dv8', cp, b)] = pt
            for b in AC8_ROWS:
                for cp in range(2):
                    pt = a8pool.tile([128, 2, N], fp8, name="a8slab", tag="a8slab")
                    for i in range(2):
                        ch = 2 * cp + i
                        nc.scalar.activation(
                            pt[:, i, :], s2t[ch][:],
                            mybir.ActivationFunctionType.Relu,
                            bias=a2t[:, ch * BSH + b : ch * BSH + b + 1],
                            scale=-1.0,
                        )
                    pair_tiles[('ac8', cp, b)] = pt
            # preload the sigmoid ACT table while the PE drains (free insurance
            # against a ~1.3us table swap on the critical tail)
            sigwarm = const.tile([32, 1], bf16, name="sigwarm", tag="sigwarm")
            nc.scalar.activation(
                sigwarm[:], ident[:, 0:1],
                mybir.ActivationFunctionType.Sigmoid,
            )
            # ---- PE pstate warmup: harmless matmuls on a zeroed scratch
            # tile while the s2t DMAs stream in, so the PE clock is ramped
            # when real work arrives
            scratch = const.tile([128, 512], bf16, name="scratch", tag="scratch")
            nc.vector.memset(scratch[:], 0)
            ps_warm = bankp.tile([32, 512], f32, name="ps_warm", tag="bank")
            for _ in range(N_WARM):
                nc.tensor.matmul(
                    ps_warm[:], scratch[:, 0:32], scratch[:],
                    start=True, stop=True, skip_group_check=True,
                )

            # ---- PE stream in estimated-ready order
            items = _schedule()
            started = False
            for idx, it in enumerate(items):
                last = idx == len(items) - 1
                start = not started
                started = True
                if it[0] == 'corr':
                    for s in range(NSEG):
                        nc.tensor.matmul(
                            psc[s][:], clhs,
                            crhs[:, SEG * s : SEG * (s + 1)],
                            start=start, stop=last, skip_group_check=True,
                        )
                elif it[0] == 'bf16':
                    _, ch, b = it
                    slab = dvb_slab[(ch, b)]
                    lhs = sgnn[:, ch, 31 - b : 63 - b]
                    for s in range(NSEG):
                        nc.tensor.matmul(
                            psc[s][:], lhs,
                            slab[:, SEG * s : SEG * (s + 1)],
                            start=start, stop=last, skip_group_check=True,
                        )
                else:
                    _, cls, cp, b = it
                    pt = pair_tiles[(cls, cp, b)]
                    k = 1 if cls == 'ac8' else 0
                    lhs = sgn8[:, k, 2 * cp : 2 * cp + 2, 31 - b : 63 - b]
                    for s in range(NSEG):
                        nc.tensor.matmul(
                            psc[s][:], lhs,
                            pt[:, :, SEG * s : SEG * (s + 1)],
                            start=start, stop=last, skip_group_check=True,
                            perf_mode=mybir.MatmulPerfMode.DoubleRow,
                        )

            # ---- sigmoid (PSUM -> SBUF, bf16) ----
            ssig = const.tile([BSH, N], bf16, name="ssig", tag="ssig")
            for s in range(NSEG):
                nc.scalar.activation(
                    ssig[:, SEG * s : SEG * (s + 1)], psc[s][:],
                    mybir.ActivationFunctionType.Sigmoid,
                )

            # ---- 16 transposes into ONE PSUM bank, one copy, final matmuls
            tpall = bankp.tile([128, NLAB * BSH], bf16, name="tpall", tag="bank")
            for k in range(NLAB):
                pk = min(128, N - 128 * k)
                nc.tensor.transpose(
                    tpall[:pk, BSH * k : BSH * k + BSH],
                    ssig[:, 128 * k : 128 * k + pk], ident[:],
                )
            sct = const.tile([128, NLAB * BSH], bf16, name="sct", tag="sct")
            nc.vector.tensor_copy(sct[:], tpall[:])
            out_ps = bankp.tile([BSH, C], f32, name="out_ps", tag="bank")
            for k in range(NLAB):
                pk = min(128, N - 128 * k)
                nc.tensor.matmul(
                    out_ps[:], sct[:pk, BSH * k : BSH * k + BSH],
                    labs[:pk, k, :],
                    start=(k == 0), stop=(k == NLAB - 1),
                )

            # ---- divide by counts, write out ----
            out_s = const.tile([BSH, C], f32, name="out_s", tag="out_s")
            nc.vector.tensor_mul(out_s[:], out_ps[:], recb[:])
            nc.sync.dma_start(out_d[:], out_s[:])

    _split_multi_waits(nc)
    return nc


def _prep_host(inputs, support_tensors, support_labels, kernel_w, kernel_b):
    import ml_dtypes

    bf16 = ml_dtypes.bfloat16
    e4m3 = ml_dtypes.float8_e4m3
    a = np.asarray(inputs, dtype=np.float32)
    S = np.asarray(support_tensors, dtype=np.float32)
    L = np.asarray(support_labels, dtype=np.float32)
    w = np.asarray(kernel_w, dtype=np.float32)
    kb = np.float32(np.asarray(kernel_b, dtype=np.float32))

    aw = 2.0 * np.abs(w)
    sgn = np.sign(w).astype(np.float32)
    s2t = np.ascontiguousarray((S * aw[None, :]).T).astype(bf16)   # [D, N]
    wS = (S @ w).astype(np.float32)                                # [N]
    wa = (a @ w).astype(np.float32)                                # [B]
    a2 = a * aw[None, :]                                           # [B, D]

    # sliding-window sign tiles: col 31 = -/+ sign chunk
    sgn_chunks = sgn.reshape(DCH, 128).T                           # [128, DCH]
    sgnn = np.zeros((128, DCH, 63), dtype=np.float32)
    sgnn[:, :, 31] = -sgn_chunks
    sgn8 = np.zeros((128, 2, DCH, 64), dtype=np.float32)
    sgn8[:, 0, :, 31] = -sgn_chunks
    sgn8[:, 1, :, 31] = sgn_chunks
    labp = np.zeros((NP, C), dtype=np.float32)
    labp[:N] = L
    labp = np.ascontiguousarray(
        labp.reshape(NLAB, 128, C).transpose(1, 0, 2)).astype(bf16)
    ident = np.eye(32, dtype=bf16)
    counts = L.sum(axis=0)
    recip = np.where(counts != 0, 1.0 / np.maximum(counts, 1e-30), 0.0)
    recb = np.broadcast_to(recip.astype(np.float32), (BSH, C)).copy()

    shared = {
        "s2t": s2t, "sgnn": sgnn.astype(bf16), "sgn8": sgn8.astype(e4m3),
        "labels": labp, "ident": ident, "recb": recb,
    }
    in_maps = []
    for c in range(NCORES):
        rows = slice(BSH * c, BSH * (c + 1))
        a2t_c = np.ascontiguousarray(
            a2[rows].T.reshape(DCH, 128, BSH))                     # [DCH,128,BSH]
        corr_c = np.empty((2, BSH + N), dtype=np.float32)
        corr_c[0, :BSH] = kb - wa[rows]
        corr_c[1, :BSH] = 1.0
        corr_c[0, BSH:] = 1.0
        corr_c[1, BSH:] = wS
        in_maps.append(dict(shared, a2t=a2t_c, corr=corr_c))
    return in_maps


def kernel(**inputs) -> np.ndarray:
    from concourse.bass_utils import run_bass_kernel_spmd

    if "nc" not in _CACHE:
        _CACHE["nc"] = _build_nc()
    nc = _CACHE["nc"]

    in_maps = _prep_host(
        inputs["inputs"], inputs["support_tensors"], inputs["support_labels"],
        inputs["kernel_w"], inputs["kernel_b"],
    )
    res = run_bass_kernel_spmd(nc, in_maps, list(range(NCORES)))
    return np.concatenate([res.results[i]["out"] for i in range(NCORES)], axis=0)
